# revision 78
# baseline (speedup 1.0000x reference)
"""BatchGAT Trainium2 kernel (Bass/Tile), data-parallel over the 8 subgraphs.

Per core (1 subgraph, n=1024 nodes, 8 heads, 2 GAT layers):
  - embedding gather via indirect DMA from the replicated 100k x 64 table
  - feature-major dataflow: xT [f, n] with features on partitions
  - attention via the separable-exp decomposition: with s_i, d_j the usual
    GAT scores and w = e^{0.8 s}, rho = e^{-0.8 d}, v = e^d, q = e^{0.2 d},
      exp(lrelu(s_i + d_j)) = p_i * (q_j + v_j * relu(w_i - rho_j)),
    p_i = e^{0.2 s_i} cancels in softmax normalization, so the masked
    numerator is  q_j*adjT[j,i] + v_j*relu(w_i - rho_j)*adjT[j,i].
    The q-term contracts directly against adjT on the PE (2 heads stacked
    per matmul at row offsets 0/64) and is folded back into the per-head
    accumulator with a shifted-identity matmul; the relu-term needs only
    TWO elementwise passes per [128, 1024] chunk, scheduled across three
    engines (variant A: DVE 4x relu-ts + DVE 2x mask-tt; B: DVE relu-ts +
    Pool mask-tt; C: DVE mask-tt of w + ACT Relu with bias -rho).  All
    exps act on O(N) row/column tensors (scalar engine).
  - normalization deferred: Z rows gathered via tiny PE transposes into
    column form, one batched fast reciprocal, transposed back, then a
    selection-matrix matmul broadcasts 1/Z to all output rows.
  - adj is transposed once per core (u8 -> bf16 convert + 64 PE transposes,
    evacuated in [128, 1024] batches) and reused by both layers.  Layer 1
    is zero-padded to fo=32 on the host so both layers share one code path.
  - layer outputs are restacked feature-major via constant selection-matrix
    matmuls (PE), head mean likewise; log_softmax in node-major space.
"""

import numpy as np

BS, N, VOCAB, EMB, FEAT = 8, 1024, 100000, 64, 64
P = 128
NCH = N // P  # 8 node chunks
H = 8
FO = 32       # per-head output features (layer 1 zero-padded to 32)
HALF = 512    # fp32 matmul free-dim limit

# wpack (f32) column layout
C_IDENT = 0            # [128,128] identity
C_B0 = 128             # 1 col, partitions 0..31
C_B1 = C_B0 + 1        # 1 col, partitions 0..15
WCOLS = C_B1 + 1
# wpackr (f32r matmul weights) column layout
C_W0 = 0               # 8 heads x 32 cols, partitions 0..127
C_AS = C_W0 + 8 * 32   # a_src: 2 layers x 8 heads x 1 col, partitions 0..31
C_AD3 = C_AS + 16      # a_dst3: 2 layers x 8 heads x 3 cols (-0.8,1,0.2)*a_dst
C_SEL = C_AD3 + 48     # 256 cols, partitions 0..7 (1/Z row select)
RCOLS = C_SEL + 256
# wpackb (bf16 matmul weights) column layout
B_IDENT = 0            # [128,128] identity
B_W1 = B_IDENT + 128   # 8 heads x 2 kchunks x 32 cols
B_MW = B_W1 + 512      # 16 cols, partitions 0..31 (head-mean /8)
B_SBLK = B_MW + 16     # 4 x 128 cols, partitions 0..31 (x1T stacking)
B_ONE = B_SBLK + 512   # 1 col, ones on partitions 0..15
B_SEL2 = B_ONE + 1     # 2 x 33 cols, shifted identities on partitions 0..96
BCOLS = B_SEL2 + 66

_CACHE = {}


def _build(zero_b0):
    import concourse.bass as bass
    import concourse.tile as tile
    from concourse import bacc, mybir
    from contextlib import ExitStack

    dt = mybir.dt
    f32 = dt.float32
    f32r = dt.float32r
    bf16 = dt.bfloat16
    A = mybir.ActivationFunctionType
    OP = mybir.AluOpType

    nc = bacc.Bacc("TRN2", target_bir_lowering=False, debug=False)

    x_d = nc.dram_tensor("x", [P, NCH * FEAT], f32, kind="ExternalInput")
    v_d = nc.dram_tensor("verts", [P, NCH], dt.int32, kind="ExternalInput")
    adj_d = nc.dram_tensor("adj", [N, N], dt.uint8, kind="ExternalInput")
    emb_d = nc.dram_tensor("emb_w", [VOCAB, EMB], f32, kind="ExternalInput")
    wp_d = nc.dram_tensor("wpack", [P, WCOLS], f32, kind="ExternalInput")
    wpb_d = nc.dram_tensor("wpackb", [P, BCOLS], dt.bfloat16, kind="ExternalInput")
    wpr_d = nc.dram_tensor("wpackr", [P, RCOLS], dt.float32r, kind="ExternalInput")
    ones_d = nc.dram_tensor("onesrow", [1, N], dt.bfloat16, kind="ExternalInput")
    # (p, c, o) layout; host reorders to [n, 16] with n = c*128+p
    out_d = nc.dram_tensor("out", [P, NCH * 16], f32, kind="ExternalOutput")

    fo = FO

    with tile.TileContext(nc) as tc, ExitStack() as ctx:
        singles = ctx.enter_context(tc.tile_pool(name="singles", bufs=1))
        stage = ctx.enter_context(tc.tile_pool(name="stage", bufs=2))
        hvpool = ctx.enter_context(tc.tile_pool(name="hvpool", bufs=1))
        big = ctx.enter_context(tc.tile_pool(name="big", bufs=2))
        epi = ctx.enter_context(tc.tile_pool(name="epi", bufs=2))
        respool = ctx.enter_context(tc.tile_pool(name="respool", bufs=8))

        # ---- input DMAs: vertex indices first so the embedding
        # gathers (software DGE on Pool) start immediately ----

        wp = singles.tile([P, WCOLS], f32, tag="wp")
        nc.sync.dma_start(out=wp[:], in_=wp_d[:, :])
        ident = wp[:, C_IDENT:C_IDENT + P]
        wpb = singles.tile([P, BCOLS], bf16, tag="wpb")
        nc.sync.dma_start(out=wpb[:], in_=wpb_d[:, :])
        identb = wpb[:, B_IDENT:B_IDENT + P]
        wpr = singles.tile([P, RCOLS], f32r, tag="wpr")
        nc.sync.dma_start(out=wpr[:], in_=wpr_d[:, :])
        x0T = singles.tile([P, N], f32r, tag="x0T")

        with tc.tile_pool(name="pstage", bufs=2, space="PSUM") as pstage, \
             tc.tile_pool(name="padj", bufs=2, space="PSUM") as padj, \
             tc.tile_pool(name="eepool", bufs=4) as eepool, \
             tc.tile_pool(name="xinpool", bufs=1) as xinpool, \
             tc.tile_pool(name="aupool", bufs=4) as aupool, \
             tc.tile_pool(name="afpool", bufs=8) as afpool:
            vts = xinpool.tile([P, NCH], dt.int32, tag="vts")
            nc.sync.dma_start(out=vts[:], in_=v_d[:, :])
            xcols = xinpool.tile([P, NCH * FEAT], f32, tag="xcols")
            nc.sync.dma_start(out=xcols[:], in_=x_d[:, :])
            pwarm = pstage.tile([P, P], f32, tag="sm")
            nc.tensor.matmul(out=pwarm[:], lhsT=ident, rhs=ident,
                             start=True, stop=True)
            pwarm2 = pstage.tile([32, 256], f32, tag="sm", name="pwarm2")
            nc.tensor.matmul(out=pwarm2[:], lhsT=wpr[:, 0:32],
                             rhs=wpr[:, 0:256], start=True, stop=True)
            for c in range(NCH):
                sl = slice(c * P, (c + 1) * P)
                ee = eepool.tile([P, EMB], f32, tag="ee")
                nc.gpsimd.indirect_dma_start(
                    out=ee[:],
                    out_offset=None,
                    in_=emb_d[:, :],
                    in_offset=bass.IndirectOffsetOnAxis(ap=vts[:, c:c + 1], axis=0),
                )
                xe = stage.tile([P, P], f32, tag="xe")
                nc.vector.tensor_copy(
                    out=xe[:, 0:FEAT], in_=xcols[:, c * FEAT:(c + 1) * FEAT]
                )
                nc.vector.tensor_copy(out=xe[:, FEAT:P], in_=ee[:])
                px = pstage.tile([P, P], f32, tag="sm")
                nc.tensor.matmul(out=px[:], lhsT=xe[:], rhs=ident,
                                 start=True, stop=True)
                nc.vector.tensor_copy(out=x0T[:, sl], in_=px[:])

            # ---- stage B: adjT bf16 [128, 8*1024]; chunk jc at cols jc*N ----
            adjT = singles.tile([P, NCH * N], bf16, tag="adjT")
            af_list = []
            for ic in range(NCH):
                au = aupool.tile([P, N], dt.uint8, tag="au")
                nc.sync.dma_start(out=au[:], in_=adj_d[ic * P:(ic + 1) * P, :])
                af = afpool.tile([P, N], bf16, tag="af")
                nc.vector.tensor_copy(out=af[:], in_=au[:])
                af_list.append(af)
            # jc-outer so adjT chunk 0 (needed first) completes first;
            # 8 transposes batch into one [128, 1024] psum, single evac.
            evac_eng = [nc.vector, nc.scalar]
            for jc in range(NCH):
                pt = padj.tile([P, N], f32, tag="pt")
                for ic in range(NCH):
                    nc.tensor.matmul(
                        out=pt[:, ic * P:(ic + 1) * P],
                        lhsT=af_list[ic][:, jc * P:(jc + 1) * P],
                        rhs=identb[:], start=True, stop=True,
                    )
                eng = evac_eng[jc % 2]
                if eng is nc.scalar:
                    nc.scalar.activation(
                        out=adjT[:, jc * N:(jc + 1) * N], in_=pt[:],
                        func=A.Identity)
                else:
                    eng.tensor_copy(out=adjT[:, jc * N:(jc + 1) * N], in_=pt[:])

        # ---- GAT layers (both padded to fo=32) ----
        x1T = [
            singles.tile([P, N], bf16, tag=f"x1T{k}", name=f"x1T{k}")
            for k in range(2)
        ]

        # hv/hq lhsT tiles are allocated once and rewritten by both
        # layers; the unused hq gap columns are zeroed a single time.
        hv_t = [[hvpool.tile([P, FO + 1], bf16, tag=f"hv_{h}_{jc}",
                             name=f"hv_{h}_{jc}")
                 for jc in range(NCH)] for h in range(H)]
        hq_t = [[hvpool.tile([P, 64 + FO + 1], bf16, tag=f"hq_{g}_{jc}",
                             name=f"hq_{g}_{jc}")
                 for jc in range(NCH)] for g in range(4)]
        for g in range(4):
            for jc in range(NCH):
                nc.gpsimd.memset(hq_t[g][jc][:, FO + 1:64], 0.0)

        xT_in = [x0T]
        msb = None
        for li in range(2):
            kch = 1 if li == 0 else 2
            hT_list = []
            wb_list = []
            evqcT = []
            evqcTn = []
            hv_all = [[None] * NCH for _ in range(H)]
            hq3 = [[None] * NCH for _ in range(4)]
            GRP = [(0, 2), (2, 4), (4, 6), (6, 8)]
            GROWS = 64 + fo + 1  # 97: 2 heads per group at offsets 0 / 64
            ou_list = []
            pq_sb = []
            zcols = singles.tile([P, H * NCH], f32, tag="zcols",
                                 name="zcols")
            # one PSUM scope for phases A / A2 / B1 / B2a: ph, pq and pat
            # accumulators share one [128, N] ring so the phases can overlap
            with tc.tile_pool(name=f"acc{li}", bufs=2, space="PSUM") as acc, \
                 tc.tile_pool(name=f"pAs{li}", bufs=1, space="PSUM") as pAs, \
                 tc.tile_pool(name=f"pAd{li}", bufs=1, space="PSUM") as pAd, \
                 tc.tile_pool(name=f"pA2{li}", bufs=1, space="PSUM") as pA2:
                # --- phase A + per-head transposes ---
                for h in range(H):
                    pha = acc.tile([P, N], f32, tag="acc")
                    ph = pha[0:fo, :]
                    for hf in range(2):
                        fs = slice(hf * HALF, (hf + 1) * HALF)
                        for k in range(kch):
                            if li == 0:
                                nc.tensor.matmul(
                                    out=ph[:, fs],
                                    lhsT=wpr[:, C_W0 + h * fo:C_W0 + (h + 1) * fo],
                                    rhs=xT_in[k][:, fs],
                                    start=(k == 0), stop=(k == kch - 1),
                                )
                            else:
                                wcol = B_W1 + (h * 2 + k) * fo
                                nc.tensor.matmul(
                                    out=ph[:, fs],
                                    lhsT=wpb[:, wcol:wcol + fo],
                                    rhs=xT_in[k][:, fs],
                                    start=(k == 0), stop=(k == kch - 1),
                                )
                    # tanh for attention scores only
                    tT = stage.tile([fo, N], f32r, tag="tT")
                    nc.scalar.activation(out=tT[:], in_=ph[:], func=A.Tanh)
                    # hT rows: 0..fo-1 h' (pre-tanh) bf16, row fo = ones
                    hT = singles.tile([fo + 1, N], bf16, tag=f"hT_{h}",
                                      name=f"hT_{h}")
                    nc.vector.tensor_copy(out=hT[0:fo, :], in_=ph[:])
                    nc.sync.dma_start(out=hT[fo:fo + 1, :], in_=ones_d[0:1, :])
                    # s broadcast to 128 partitions, then w = exp(0.8 s)
                    acol = wpr[0:fo, C_AS + li * H + h: C_AS + li * H + h + 1]
                    psb = pAs.tile([P, N], f32, tag="psb")
                    for hf in range(2):
                        fs = slice(hf * HALF, (hf + 1) * HALF)
                        nc.tensor.matmul(
                            out=psb[:, fs],
                            lhsT=acol.to_broadcast([fo, P]),
                            rhs=tT[:, fs],
                            start=True, stop=True,
                        )
                    wb = singles.tile([P, N], bf16, tag=f"wb_{h}",
                                      name=f"wb_{h}")
                    nc.scalar.activation(out=wb[:], in_=psb[:], func=A.Exp,
                                         scale=0.8)
                    wb_list.append(wb)
                    # d columns: (-0.8 d, d, 0.2 d) per chunk, batched exp
                    adcol = C_AD3 + (li * H + h) * 3
                    pd3 = pAd.tile([P, 24], f32, tag="pd3")
                    for jc in range(NCH):
                        nc.tensor.matmul(
                            out=pd3[:, 3 * jc:3 * jc + 3],
                            lhsT=tT[:, jc * P:(jc + 1) * P].bitcast(f32),
                            rhs=wpr[0:fo, adcol:adcol + 3].bitcast(f32),
                            start=True, stop=True,
                        )
                    ev = singles.tile([P, 24], f32, tag=f"evc_{h}",
                                      name=f"evc_{h}")
                    nc.scalar.activation(out=ev[:], in_=pd3[:], func=A.Exp)
                    evn = singles.tile([P, 24], f32, tag=f"evn_{h}",
                                       name=f"evn_{h}")
                    nc.vector.tensor_scalar(
                        out=evn[:], in0=ev[:], scalar1=-1.0, scalar2=None,
                        op0=OP.mult,
                    )
                    evqcT.append(ev)
                    evqcTn.append(evn)
                    hT_list.append(hT)
                    # per-chunk transpose of [h'; ones], v-scaled lhsT
                    for jc in range(NCH):
                        ptr = pA2.tile([P, fo + 1], f32, tag="ptr")
                        nc.tensor.matmul(
                            out=ptr[:],
                            lhsT=hT[:, jc * P:(jc + 1) * P],
                            rhs=identb[0:fo + 1, 0:fo + 1],
                            start=True, stop=True,
                        )
                        hv = hv_t[h][jc]
                        nc.vector.tensor_scalar(
                            out=hv[:], in0=ptr[:],
                            scalar1=ev[:, 3 * jc + 1:3 * jc + 2], scalar2=None,
                            op0=OP.mult,
                        )
                        hv_all[h][jc] = hv
                    # q-term group lhsT ready once both heads of the pair
                    # are done: q = rho * v, so q-scaled ha = rho * hv
                    if h % 2 == 1:
                        g = h // 2
                        for jc in range(NCH):
                            hq = hq_t[g][jc]
                            for hh in (h - 1, h):
                                off = (hh % 2) * 64
                                nc.vector.tensor_scalar(
                                    out=hq[:, off:off + fo + 1],
                                    in0=hv_all[hh][jc][:],
                                    scalar1=evqcT[hh][:, 3 * jc:3 * jc + 1],
                                    scalar2=None, op0=OP.mult,
                                )
                            hq3[g][jc] = hq
                # --- phases B1+B2a interleaved: pq sweep for group g, then
                # relu sweeps for its two heads; the next group's pq matmuls
                # overlap the current heads' elementwise masks ---
                # mask-variant schedule per (h, jc): A = all-DVE (ts 4x +
                # tt 2x), B = DVE relu-ts + Pool tt-mult mask, C = DVE
                # tt-mask of w then ACT Relu with bias -rho:
                # relu(w*adj - rho) == relu(w - rho)*adj since rho > 0.
                VAR = "CACBCACBCACBCABC"
                for g in range(4):
                    pqa = acc.tile([P, N], f32, tag="acc")
                    pqp = pqa[0:GROWS, :]
                    for jc in range(NCH):
                        for hf in range(2):
                            nc.tensor.matmul(
                                out=pqp[:, hf * HALF:(hf + 1) * HALF],
                                lhsT=hq3[g][jc][:],
                                rhs=adjT[:, jc * N + hf * HALF:
                                         jc * N + (hf + 1) * HALF],
                                start=(jc == 0), stop=(jc == NCH - 1),
                            )
                    sb = singles.tile([GROWS, N], bf16, tag=f"pqsb_{g}",
                                      name=f"pqsb_{g}")
                    nc.scalar.activation(out=sb[:], in_=pqp[:], func=A.Identity)
                    pq_sb.append(sb)
                    for h in (2 * g, 2 * g + 1):
                        wb = wb_list[h]
                        ev = evqcT[h]
                        evn = evqcTn[h]
                        pata = acc.tile([P, N], f32, tag="acc")
                        pat = pata[0:fo + 1, :]
                        for jc in range(NCH):
                            v = VAR[(h * NCH + jc) % 16]
                            adjc = adjT[:, jc * N:(jc + 1) * N]
                            rho = ev[:, 3 * jc:3 * jc + 1]
                            nrho = evn[:, 3 * jc:3 * jc + 1]
                            mk = big.tile([P, N], bf16, tag="mk", bufs=7)
                            if v == "C":
                                t0 = big.tile([P, N], bf16, tag="rl", bufs=8)
                                nc.vector.tensor_tensor(
                                    out=t0[:], in0=wb[:], in1=adjc, op=OP.mult,
                                )
                                nc.scalar.activation(out=mk[:], in_=t0[:],
                                                     func=A.Relu, bias=nrho)
                            else:
                                rl = big.tile([P, N], bf16, tag="rl", bufs=8)
                                nc.vector.tensor_scalar(
                                    out=rl[:], in0=wb[:],
                                    scalar1=rho, scalar2=0.0,
                                    op0=OP.subtract, op1=OP.max,
                                )
                                if v == "A":
                                    nc.vector.tensor_tensor(
                                        out=mk[:], in0=rl[:], in1=adjc,
                                        op=OP.mult,
                                    )
                                else:
                                    nc.gpsimd.tensor_tensor(
                                        out=mk[:], in0=rl[:], in1=adjc,
                                        op=OP.mult,
                                    )
                            for hf in range(2):
                                fs = slice(hf * HALF, (hf + 1) * HALF)
                                nc.tensor.matmul(
                                    out=pat[:, fs],
                                    lhsT=hv_all[h][jc][:],
                                    rhs=mk[:, fs],
                                    start=(jc == 0), stop=False,
                                )
                        # fold the q-term into pat on the PE via a
                        # shifted-identity lhsT, then evacuate on ACT
                        k = h % 2
                        sel = wpb[0:GROWS, B_SEL2 + k * (fo + 1):
                                  B_SEL2 + (k + 1) * (fo + 1)]
                        for hf in range(2):
                            fs = slice(hf * HALF, (hf + 1) * HALF)
                            nc.tensor.matmul(
                                out=pat[:, fs], lhsT=sel, rhs=pq_sb[g][:, fs],
                                start=False, stop=True,
                            )
                        ou = singles.tile([fo + 1, N], bf16, tag=f"ou_{h}",
                                          name=f"ou_{h}")
                        nc.scalar.activation(out=ou[:], in_=pat[:],
                                             func=A.Identity)
                        ou_list.append(ou)
                        # gather this head's Z row into column form
                        # (zcols col 8h+c <-> (head h, chunk c))
                        pzh3 = pAd.tile([P, 24], f32, tag="pd3")
                        pzh = pzh3[:, 0:H]
                        for c in range(NCH):
                            nc.tensor.matmul(
                                out=pzh[:, c:c + 1],
                                lhsT=ou[fo:fo + 1, c * P:(c + 1) * P],
                                rhs=identb[fo:fo + 1, fo:fo + 1],
                                start=True, stop=True,
                            )
                        nc.vector.tensor_copy(
                            out=zcols[:, H * h:H * h + H], in_=pzh[:]
                        )
            # --- phase B2b/B2c: Z -> 1/Z, normalize, activation epilogue ---
            rall = singles.tile([H, N], f32r, tag="rall", name="rall")
            xr_list = []
            pm = None
            with tc.tile_pool(name=f"pBz{li}", bufs=2, space="PSUM") as pBz, \
                 tc.tile_pool(name=f"pBr{li}", bufs=2, space="PSUM") as pBr, \
                 tc.tile_pool(name=f"pBm{li}", bufs=1, space="PSUM") as pBm:
                rcols = singles.tile([P, H * NCH], f32, tag="rcols",
                                     name="rcols")
                rscr = singles.tile([P, H * NCH], f32, tag="rscr",
                                    name="rscr")
                nc.vector.reciprocal_approx_accurate(
                    out=rcols[:], in_=zcols[:], scratch=rscr[:]
                )
                for c in range(NCH):
                    prr = pBz.tile([H, P], f32, tag="rr")
                    nc.tensor.matmul(
                        out=prr[:],
                        lhsT=rcols[:, c:c + (H - 1) * H + 1:H], rhs=ident,
                        start=True, stop=True,
                    )
                    nc.vector.tensor_copy(
                        out=rall[:, c * P:(c + 1) * P], in_=prr[:]
                    )
                def emit_prb(h):
                    prb = pBr.tile([fo, N], f32, tag="prb")
                    for hf in range(2):
                        fs = slice(hf * HALF, (hf + 1) * HALF)
                        nc.tensor.matmul(
                            out=prb[:, fs],
                            lhsT=wpr[0:H, C_SEL + h * fo: C_SEL + (h + 1) * fo],
                            rhs=rall[:, fs],
                            start=True, stop=True,
                        )
                    return prb

                # lookahead: head h+1's 1/Z broadcast is emitted before head
                # h's dependent ops so the PE queue never stalls on y_h
                prb_next = emit_prb(0)
                for h in range(H):
                    prb = prb_next
                    if h + 1 < H:
                        prb_next = emit_prb(h + 1)
                    y = epi.tile([fo, N], bf16, tag="y", bufs=3)
                    if li == 0:
                        # l0: stage 1/Z through ACT so the DVE multiply
                        # runs in 2x mode (DVE is saturated here)
                        prs = epi.tile([fo, N], bf16, tag="prs", bufs=2)
                        nc.scalar.activation(out=prs[:], in_=prb[:],
                                             func=A.Identity)
                        nc.vector.tensor_tensor(
                            out=y[:], in0=ou_list[h][0:fo, :], in1=prs[:],
                            op=OP.mult
                        )
                    else:
                        # l1 tail: shorter chain, read prb from PSUM
                        nc.vector.tensor_tensor(
                            out=y[:], in0=ou_list[h][0:fo, :], in1=prb[:],
                            op=OP.mult
                        )
                    if li == 0:
                        # x1 rows = elu(y + b0)
                        if not zero_b0:
                            yb = epi.tile([fo, N], bf16, tag="yb")
                            nc.vector.tensor_scalar(
                                out=yb[:], in0=y[:],
                                scalar1=wp[0:fo, C_B0:C_B0 + 1],
                                scalar2=None, op0=OP.add,
                            )
                            y = yb
                        m = epi.tile([fo, N], bf16, tag="m", bufs=2)
                        nc.vector.tensor_scalar(
                            out=m[:], in0=y[:], scalar1=0.0, scalar2=None,
                            op0=OP.min
                        )
                        e = epi.tile([fo, N], bf16, tag="e", bufs=2)
                        nc.scalar.activation(out=e[:], in_=m[:], func=A.Exp)
                        xr = epi.tile([fo, N], bf16, tag="xr", bufs=8,
                                      name=f"xr{h}")
                        nc.vector.scalar_tensor_tensor(
                            out=xr[:], in0=e[:], scalar=-1.0, in1=y[:],
                            op0=OP.add, op1=OP.max,
                        )
                        xr_list.append(xr)
                        # restack half as soon as its 4 heads are done
                        if h % 4 == 3:
                            k = h // 4
                            px1 = pBm.tile([P, N], f32, tag="mx",
                                           name=f"px1_{k}")
                            for hf in range(2):
                                fs = slice(hf * HALF, (hf + 1) * HALF)
                                for j in range(4):
                                    nc.tensor.matmul(
                                        out=px1[:, fs],
                                        lhsT=wpb[0:fo, B_SBLK + j * P:
                                                 B_SBLK + (j + 1) * P],
                                        rhs=xr_list[k * 4 + j][:, fs],
                                        start=(j == 0), stop=(j == 3),
                                    )
                            nc.scalar.activation(out=x1T[k][:], in_=px1[:],
                                                 func=A.Identity)
                    else:
                        # head-mean accumulation: pm += mw^T @ y
                        if pm is None:
                            pm = pBm.tile([16, N], f32, tag="mx")
                        for hf in range(2):
                            fs = slice(hf * HALF, (hf + 1) * HALF)
                            nc.tensor.matmul(
                                out=pm[:, fs],
                                lhsT=wpb[0:fo, B_MW:B_MW + 16],
                                rhs=y[:, fs],
                                start=(h == 0), stop=(h == H - 1),
                            )
                if li == 0:
                    xT_in = x1T
                else:
                    # rows 0:16 = pm + b1; row 32 later holds logZ
                    msbt = singles.tile([33, N], f32, tag="msb")
                    msb = msbt[0:16, :]
                    nc.vector.tensor_scalar(
                        out=msb[:], in0=pm[:], scalar1=wp[0:16, C_B1:C_B1 + 1],
                        scalar2=None, op0=OP.add,
                    )

        # ---- log_softmax over the 16 features ----
        # feature-major: one exp + ones-column colsum (PE) + one ln, then
        # per-chunk transposes carry both msb^T and logZ^T; final subtract
        # is a per-partition-scalar tensor_scalar. Scores are bounded, so
        # no max-subtraction is needed.
        with tc.tile_pool(name="pfin", bufs=2, space="PSUM") as pfin, \
             tc.tile_pool(name="pfz", bufs=1, space="PSUM") as pfz, \
             tc.tile_pool(name="finsb", bufs=1) as finsb:
            pexp = finsb.tile([16, N], bf16, tag="pexp")
            nc.scalar.activation(out=pexp[:], in_=msb[:], func=A.Exp)
            pzs = pfz.tile([1, N], f32, tag="zs")
            for hf in range(2):
                fs = slice(hf * HALF, (hf + 1) * HALF)
                nc.tensor.matmul(
                    out=pzs[:, fs], lhsT=wpb[0:16, B_ONE:B_ONE + 1],
                    rhs=pexp[:, fs], start=True, stop=True,
                )
            lgz = msbt[32:33, :]
            nc.scalar.activation(out=lgz[:], in_=pzs[:], func=A.Ln)
            outall = finsb.tile([P, NCH * 16], f32, tag="outall")
            for ic in range(NCH):
                pf = pfin.tile([P, 17], f32, tag="sm")
                nc.tensor.matmul(
                    out=pf[:, 0:16], lhsT=msb[:, ic * P:(ic + 1) * P],
                    rhs=wp[0:16, 0:16],
                    start=True, stop=True,
                )
                nc.tensor.matmul(
                    out=pf[:, 16:17], lhsT=lgz[:, ic * P:(ic + 1) * P],
                    rhs=wp[32:33, 32:33],
                    start=True, stop=True,
                )
                res = respool.tile([P, 17], f32, tag="res")
                nc.vector.tensor_copy(out=res[:], in_=pf[:])
                nc.vector.tensor_scalar(
                    out=outall[:, 16 * ic:16 * (ic + 1)], in0=res[:, 0:16],
                    scalar1=res[:, 16:17],
                    scalar2=None, op0=OP.subtract,
                )
            # single contiguous output DMA; host undoes the layout
            nc.sync.dma_start(out=out_d[:, :], in_=outall[:])

    nc.compile()
    return nc


def _make_wpack(inputs):
    f32 = np.float32
    import ml_dtypes
    wpack = np.zeros((P, WCOLS), f32)
    wpack[:, C_IDENT:C_IDENT + P] = np.eye(P, dtype=f32)
    wpack[0:FO, C_B0] = np.asarray(inputs["b0"], f32).reshape(FO)
    wpack[0:16, C_B1] = np.asarray(inputs["b1"], f32).reshape(16)

    wpr = np.zeros((P, RCOLS), f32)
    w0 = np.asarray(inputs["w0"], f32)
    for h in range(H):
        wpr[:, C_W0 + h * FO: C_W0 + (h + 1) * FO] = w0[h]
    a_src0 = np.asarray(inputs["a_src0"], f32)[..., 0]  # [8, 32]
    a_dst0 = np.asarray(inputs["a_dst0"], f32)[..., 0]
    a_src1 = np.asarray(inputs["a_src1"], f32)[..., 0]  # [8, 16]
    a_dst1 = np.asarray(inputs["a_dst1"], f32)[..., 0]
    for h in range(H):
        wpr[0:FO, C_AS + h] = a_src0[h]
        wpr[0:16, C_AS + H + h] = a_src1[h]
        for k, c in enumerate((-0.8, 1.0, 0.2)):
            wpr[0:FO, C_AD3 + 3 * h + k] = c * a_dst0[h]
            wpr[0:16, C_AD3 + 3 * (H + h) + k] = c * a_dst1[h]
    wpr[0:H, C_SEL:C_SEL + H * FO] = np.kron(
        np.eye(H, dtype=f32), np.ones((1, FO), f32)
    )

    wpb = np.zeros((P, BCOLS), f32)
    wpb[:, B_IDENT:B_IDENT + P] = np.eye(P, dtype=f32)
    w1 = np.asarray(inputs["w1"], f32)  # [8, 256, 16]
    for h in range(H):
        for k in range(2):
            blk = np.zeros((P, FO), f32)
            blk[:, :16] = w1[h, k * P:(k + 1) * P, :]
            wpb[:, B_W1 + (h * 2 + k) * FO: B_W1 + (h * 2 + k + 1) * FO] = blk
    wpb[0:16, B_MW:B_MW + 16] = np.eye(16, dtype=f32) / 8.0
    wpb[0:16, B_ONE] = 1.0
    for k in range(2):
        blk = np.zeros((P, FO + 1), f32)
        for p in range(FO + 1):
            blk[64 * k + p, p] = 1.0
        wpb[:, B_SEL2 + k * (FO + 1): B_SEL2 + (k + 1) * (FO + 1)] = blk
    for j in range(4):
        wpb[0:FO, B_SBLK + j * P: B_SBLK + (j + 1) * P] = np.eye(
            FO, P, k=j * FO, dtype=f32
        )
    wpb = wpb.astype(ml_dtypes.bfloat16)
    return wpack, wpr, wpb


def _prep_inputs(inputs):
    x = np.asarray(inputs["x"], np.float32)
    verts = np.asarray(inputs["vertices"]).astype(np.int32)
    adj = np.asarray(inputs["adj"]).astype(np.uint8)
    emb_w = np.ascontiguousarray(np.asarray(inputs["emb_w"], np.float32))
    wpack, wpr, wpb = _make_wpack(inputs)
    wpack = np.ascontiguousarray(wpack)
    wpr = np.ascontiguousarray(wpr)
    wpb = np.ascontiguousarray(wpb)
    import ml_dtypes
    onesrow = np.ascontiguousarray(np.ones((1, N), dtype=ml_dtypes.bfloat16))
    in_maps = []
    for c in range(BS):
        in_maps.append({
            "x": np.ascontiguousarray(
                x[c].reshape(NCH, P, FEAT).transpose(1, 0, 2).reshape(P, NCH * FEAT)
            ),
            "verts": np.ascontiguousarray(
                verts[c].reshape(NCH, P).T
            ),
            "adj": np.ascontiguousarray(adj[c]),
            "emb_w": emb_w,
            "wpack": wpack,
            "wpackb": wpb,
            "wpackr": wpr,
            "onesrow": onesrow,
        })
    zero_b0 = bool(np.all(np.asarray(inputs["b0"]) == 0))
    return in_maps, zero_b0


def _run(inputs, trace=False):
    from concourse.bass_utils import run_bass_kernel_spmd

    in_maps, zero_b0 = _prep_inputs(inputs)
    key = ("prog", zero_b0)
    if key not in _CACHE:
        _CACHE[key] = _build(zero_b0)
    nc = _CACHE[key]
    res = run_bass_kernel_spmd(
        nc, in_maps, list(range(BS)), trace=trace
    )
    outs = []
    for c in range(BS):
        o = np.asarray(res.results[c]["out"]).reshape(P, NCH, 16)
        outs.append(o.transpose(1, 0, 2).reshape(N, 16))
    out = np.stack(outs, axis=0)
    return out.astype(np.float32), res


def kernel(**inputs):
    out, _ = _run(inputs, trace=False)
    return out


# revision 79
# speedup vs baseline: 1.0037x; 1.0037x over previous
"""BatchGAT Trainium2 kernel (Bass/Tile), data-parallel over the 8 subgraphs.

Per core (1 subgraph, n=1024 nodes, 8 heads, 2 GAT layers):
  - embedding gather via indirect DMA from the replicated 100k x 64 table
  - feature-major dataflow: xT [f, n] with features on partitions
  - attention via the separable-exp decomposition: with s_i, d_j the usual
    GAT scores and w = e^{0.8 s}, rho = e^{-0.8 d}, v = e^d, q = e^{0.2 d},
      exp(lrelu(s_i + d_j)) = p_i * (q_j + v_j * relu(w_i - rho_j)),
    p_i = e^{0.2 s_i} cancels in softmax normalization, so the masked
    numerator is  q_j*adjT[j,i] + v_j*relu(w_i - rho_j)*adjT[j,i].
    The q-term contracts directly against adjT on the PE (2 heads stacked
    per matmul at row offsets 0/64) and is folded back into the per-head
    accumulator with a shifted-identity matmul; the relu-term needs only
    TWO elementwise passes per [128, 1024] chunk, scheduled across three
    engines (variant A: DVE 4x relu-ts + DVE 2x mask-tt; B: DVE relu-ts +
    Pool mask-tt; C: DVE mask-tt of w + ACT Relu with bias -rho).  All
    exps act on O(N) row/column tensors (scalar engine).
  - normalization deferred: Z rows gathered via tiny PE transposes into
    column form, one batched fast reciprocal, transposed back, then a
    selection-matrix matmul broadcasts 1/Z to all output rows.
  - adj is transposed once per core (u8 -> bf16 convert + 64 PE transposes,
    evacuated in [128, 1024] batches) and reused by both layers.  Layer 1
    is zero-padded to fo=32 on the host so both layers share one code path.
  - layer outputs are restacked feature-major via constant selection-matrix
    matmuls (PE), head mean likewise; log_softmax in node-major space.
"""

import numpy as np

BS, N, VOCAB, EMB, FEAT = 8, 1024, 100000, 64, 64
P = 128
NCH = N // P  # 8 node chunks
H = 8
FO = 32       # per-head output features (layer 1 zero-padded to 32)
HALF = 512    # fp32 matmul free-dim limit

# wpack (f32) column layout
C_IDENT = 0            # [128,128] identity
C_B0 = 128             # 1 col, partitions 0..31
C_B1 = C_B0 + 1        # 1 col, partitions 0..15
WCOLS = C_B1 + 1
# wpackr (f32r matmul weights) column layout
C_W0 = 0               # 8 heads x 32 cols, partitions 0..127
C_AS = C_W0 + 8 * 32   # a_src: 2 layers x 8 heads x 1 col, partitions 0..31
C_AD3 = C_AS + 16      # a_dst3: 2 layers x 8 heads x 3 cols (-0.8,1,0.2)*a_dst
C_SEL = C_AD3 + 48     # 256 cols, partitions 0..7 (1/Z row select)
RCOLS = C_SEL + 256
# wpackb (bf16 matmul weights) column layout
B_IDENT = 0            # [128,128] identity
B_W1 = B_IDENT + 128   # 8 heads x 2 kchunks x 32 cols
B_MW = B_W1 + 512      # 16 cols, partitions 0..31 (head-mean /8)
B_SBLK = B_MW + 16     # 4 x 128 cols, partitions 0..31 (x1T stacking)
B_ONE = B_SBLK + 512   # 1 col, ones on partitions 0..15
B_SEL2 = B_ONE + 1     # 2 x 33 cols, shifted identities on partitions 0..96
BCOLS = B_SEL2 + 66

_CACHE = {}


def _build(zero_b0):
    import concourse.bass as bass
    import concourse.tile as tile
    from concourse import bacc, mybir
    from contextlib import ExitStack

    dt = mybir.dt
    f32 = dt.float32
    f32r = dt.float32r
    bf16 = dt.bfloat16
    A = mybir.ActivationFunctionType
    OP = mybir.AluOpType

    nc = bacc.Bacc("TRN2", target_bir_lowering=False, debug=False)

    x_d = nc.dram_tensor("x", [P, NCH * FEAT], f32, kind="ExternalInput")
    v_d = nc.dram_tensor("verts", [P, NCH], dt.int32, kind="ExternalInput")
    adj_d = nc.dram_tensor("adj", [N, N], dt.uint8, kind="ExternalInput")
    emb_d = nc.dram_tensor("emb_w", [VOCAB, EMB], f32, kind="ExternalInput")
    wp_d = nc.dram_tensor("wpack", [P, WCOLS], f32, kind="ExternalInput")
    wpb_d = nc.dram_tensor("wpackb", [P, BCOLS], dt.bfloat16, kind="ExternalInput")
    wpr_d = nc.dram_tensor("wpackr", [P, RCOLS], dt.float32r, kind="ExternalInput")
    ones_d = nc.dram_tensor("onesrow", [1, N], dt.bfloat16, kind="ExternalInput")
    # (p, c, o) layout; host reorders to [n, 16] with n = c*128+p
    out_d = nc.dram_tensor("out", [P, NCH * 16], f32, kind="ExternalOutput")

    fo = FO

    with tile.TileContext(nc) as tc, ExitStack() as ctx:
        singles = ctx.enter_context(tc.tile_pool(name="singles", bufs=1))
        stage = ctx.enter_context(tc.tile_pool(name="stage", bufs=2))
        hvpool = ctx.enter_context(tc.tile_pool(name="hvpool", bufs=1))
        big = ctx.enter_context(tc.tile_pool(name="big", bufs=2))
        epi = ctx.enter_context(tc.tile_pool(name="epi", bufs=2))
        respool = ctx.enter_context(tc.tile_pool(name="respool", bufs=8))

        # ---- input DMAs: vertex indices first so the embedding
        # gathers (software DGE on Pool) start immediately ----

        wp = singles.tile([P, WCOLS], f32, tag="wp")
        nc.sync.dma_start(out=wp[:], in_=wp_d[:, :])
        ident = wp[:, C_IDENT:C_IDENT + P]
        wpb = singles.tile([P, BCOLS], bf16, tag="wpb")
        nc.sync.dma_start(out=wpb[:], in_=wpb_d[:, :])
        identb = wpb[:, B_IDENT:B_IDENT + P]
        wpr = singles.tile([P, RCOLS], f32r, tag="wpr")
        nc.sync.dma_start(out=wpr[:], in_=wpr_d[:, :])
        x0T = singles.tile([P, N], f32r, tag="x0T")

        with tc.tile_pool(name="pstage", bufs=2, space="PSUM") as pstage, \
             tc.tile_pool(name="padj", bufs=2, space="PSUM") as padj, \
             tc.tile_pool(name="eepool", bufs=4) as eepool, \
             tc.tile_pool(name="xinpool", bufs=1) as xinpool, \
             tc.tile_pool(name="aupool", bufs=4) as aupool, \
             tc.tile_pool(name="afpool", bufs=8) as afpool:
            vts = xinpool.tile([P, NCH], dt.int32, tag="vts")
            nc.sync.dma_start(out=vts[:], in_=v_d[:, :])
            xcols = xinpool.tile([P, NCH * FEAT], f32, tag="xcols")
            nc.sync.dma_start(out=xcols[:], in_=x_d[:, :])
            pwarm = pstage.tile([P, P], f32, tag="sm")
            nc.tensor.matmul(out=pwarm[:], lhsT=ident, rhs=ident,
                             start=True, stop=True)
            pwarm2 = pstage.tile([32, 256], f32, tag="sm", name="pwarm2")
            nc.tensor.matmul(out=pwarm2[:], lhsT=wpr[:, 0:32],
                             rhs=wpr[:, 0:256], start=True, stop=True)
            for c in range(NCH):
                sl = slice(c * P, (c + 1) * P)
                ee = eepool.tile([P, EMB], f32, tag="ee")
                nc.gpsimd.indirect_dma_start(
                    out=ee[:],
                    out_offset=None,
                    in_=emb_d[:, :],
                    in_offset=bass.IndirectOffsetOnAxis(ap=vts[:, c:c + 1], axis=0),
                )
                xe = stage.tile([P, P], f32, tag="xe")
                nc.vector.tensor_copy(
                    out=xe[:, 0:FEAT], in_=xcols[:, c * FEAT:(c + 1) * FEAT]
                )
                nc.vector.tensor_copy(out=xe[:, FEAT:P], in_=ee[:])
                px = pstage.tile([P, P], f32, tag="sm")
                nc.tensor.matmul(out=px[:], lhsT=xe[:], rhs=ident,
                                 start=True, stop=True)
                nc.vector.tensor_copy(out=x0T[:, sl], in_=px[:])

            # ---- stage B: adjT bf16 [128, 8*1024]; chunk jc at cols jc*N ----
            adjT = singles.tile([P, NCH * N], bf16, tag="adjT")
            af_list = []
            for ic in range(NCH):
                au = aupool.tile([P, N], dt.uint8, tag="au")
                nc.sync.dma_start(out=au[:], in_=adj_d[ic * P:(ic + 1) * P, :])
                af = afpool.tile([P, N], bf16, tag="af")
                nc.scalar.activation(out=af[:], in_=au[:], func=A.Identity)
                af_list.append(af)
            # jc-outer so adjT chunk 0 (needed first) completes first;
            # 8 transposes batch into one [128, 1024] psum, single evac.
            evac_eng = [nc.vector, nc.scalar]
            for jc in range(NCH):
                pt = padj.tile([P, N], f32, tag="pt")
                for ic in range(NCH):
                    nc.tensor.matmul(
                        out=pt[:, ic * P:(ic + 1) * P],
                        lhsT=af_list[ic][:, jc * P:(jc + 1) * P],
                        rhs=identb[:], start=True, stop=True,
                    )
                eng = evac_eng[jc % 2]
                if eng is nc.scalar:
                    nc.scalar.activation(
                        out=adjT[:, jc * N:(jc + 1) * N], in_=pt[:],
                        func=A.Identity)
                else:
                    eng.tensor_copy(out=adjT[:, jc * N:(jc + 1) * N], in_=pt[:])

        # ---- GAT layers (both padded to fo=32) ----
        x1T = [
            singles.tile([P, N], bf16, tag=f"x1T{k}", name=f"x1T{k}")
            for k in range(2)
        ]

        # hv/hq lhsT tiles are allocated once and rewritten by both
        # layers; the unused hq gap columns are zeroed a single time.
        hv_t = [[hvpool.tile([P, FO + 1], bf16, tag=f"hv_{h}_{jc}",
                             name=f"hv_{h}_{jc}")
                 for jc in range(NCH)] for h in range(H)]
        hq_t = [[hvpool.tile([P, 64 + FO + 1], bf16, tag=f"hq_{g}_{jc}",
                             name=f"hq_{g}_{jc}")
                 for jc in range(NCH)] for g in range(4)]
        for g in range(4):
            for jc in range(NCH):
                nc.gpsimd.memset(hq_t[g][jc][:, FO + 1:64], 0.0)

        xT_in = [x0T]
        msb = None
        for li in range(2):
            kch = 1 if li == 0 else 2
            hT_list = []
            wb_list = []
            evqcT = []
            evqcTn = []
            hv_all = [[None] * NCH for _ in range(H)]
            hq3 = [[None] * NCH for _ in range(4)]
            GRP = [(0, 2), (2, 4), (4, 6), (6, 8)]
            GROWS = 64 + fo + 1  # 97: 2 heads per group at offsets 0 / 64
            ou_list = []
            pq_sb = []
            zcols = singles.tile([P, H * NCH], f32, tag="zcols",
                                 name="zcols")
            # one PSUM scope for phases A / A2 / B1 / B2a: ph, pq and pat
            # accumulators share one [128, N] ring so the phases can overlap
            with tc.tile_pool(name=f"acc{li}", bufs=2, space="PSUM") as acc, \
                 tc.tile_pool(name=f"pAs{li}", bufs=1, space="PSUM") as pAs, \
                 tc.tile_pool(name=f"pAd{li}", bufs=1, space="PSUM") as pAd, \
                 tc.tile_pool(name=f"pA2{li}", bufs=1, space="PSUM") as pA2:
                # --- phase A + per-head transposes ---
                for h in range(H):
                    pha = acc.tile([P, N], f32, tag="acc")
                    ph = pha[0:fo, :]
                    for hf in range(2):
                        fs = slice(hf * HALF, (hf + 1) * HALF)
                        for k in range(kch):
                            if li == 0:
                                nc.tensor.matmul(
                                    out=ph[:, fs],
                                    lhsT=wpr[:, C_W0 + h * fo:C_W0 + (h + 1) * fo],
                                    rhs=xT_in[k][:, fs],
                                    start=(k == 0), stop=(k == kch - 1),
                                )
                            else:
                                wcol = B_W1 + (h * 2 + k) * fo
                                nc.tensor.matmul(
                                    out=ph[:, fs],
                                    lhsT=wpb[:, wcol:wcol + fo],
                                    rhs=xT_in[k][:, fs],
                                    start=(k == 0), stop=(k == kch - 1),
                                )
                    # tanh for attention scores only
                    tT = stage.tile([fo, N], f32r, tag="tT")
                    nc.scalar.activation(out=tT[:], in_=ph[:], func=A.Tanh)
                    # hT rows: 0..fo-1 h' (pre-tanh) bf16, row fo = ones
                    hT = singles.tile([fo + 1, N], bf16, tag=f"hT_{h}",
                                      name=f"hT_{h}")
                    nc.vector.tensor_copy(out=hT[0:fo, :], in_=ph[:])
                    nc.sync.dma_start(out=hT[fo:fo + 1, :], in_=ones_d[0:1, :])
                    # s broadcast to 128 partitions, then w = exp(0.8 s)
                    acol = wpr[0:fo, C_AS + li * H + h: C_AS + li * H + h + 1]
                    psb = pAs.tile([P, N], f32, tag="psb")
                    for hf in range(2):
                        fs = slice(hf * HALF, (hf + 1) * HALF)
                        nc.tensor.matmul(
                            out=psb[:, fs],
                            lhsT=acol.to_broadcast([fo, P]),
                            rhs=tT[:, fs],
                            start=True, stop=True,
                        )
                    wb = singles.tile([P, N], bf16, tag=f"wb_{h}",
                                      name=f"wb_{h}")
                    nc.scalar.activation(out=wb[:], in_=psb[:], func=A.Exp,
                                         scale=0.8)
                    wb_list.append(wb)
                    # d columns: (-0.8 d, d, 0.2 d) per chunk, batched exp
                    adcol = C_AD3 + (li * H + h) * 3
                    pd3 = pAd.tile([P, 24], f32, tag="pd3")
                    for jc in range(NCH):
                        nc.tensor.matmul(
                            out=pd3[:, 3 * jc:3 * jc + 3],
                            lhsT=tT[:, jc * P:(jc + 1) * P].bitcast(f32),
                            rhs=wpr[0:fo, adcol:adcol + 3].bitcast(f32),
                            start=True, stop=True,
                        )
                    ev = singles.tile([P, 24], f32, tag=f"evc_{h}",
                                      name=f"evc_{h}")
                    nc.scalar.activation(out=ev[:], in_=pd3[:], func=A.Exp)
                    evn = singles.tile([P, 24], f32, tag=f"evn_{h}",
                                       name=f"evn_{h}")
                    nc.vector.tensor_scalar(
                        out=evn[:], in0=ev[:], scalar1=-1.0, scalar2=None,
                        op0=OP.mult,
                    )
                    evqcT.append(ev)
                    evqcTn.append(evn)
                    hT_list.append(hT)
                    # per-chunk transpose of [h'; ones], v-scaled lhsT
                    for jc in range(NCH):
                        ptr = pA2.tile([P, fo + 1], f32, tag="ptr")
                        nc.tensor.matmul(
                            out=ptr[:],
                            lhsT=hT[:, jc * P:(jc + 1) * P],
                            rhs=identb[0:fo + 1, 0:fo + 1],
                            start=True, stop=True,
                        )
                        hv = hv_t[h][jc]
                        nc.vector.tensor_scalar(
                            out=hv[:], in0=ptr[:],
                            scalar1=ev[:, 3 * jc + 1:3 * jc + 2], scalar2=None,
                            op0=OP.mult,
                        )
                        hv_all[h][jc] = hv
                    # q-term group lhsT ready once both heads of the pair
                    # are done: q = rho * v, so q-scaled ha = rho * hv
                    if h % 2 == 1:
                        g = h // 2
                        for jc in range(NCH):
                            hq = hq_t[g][jc]
                            for hh in (h - 1, h):
                                off = (hh % 2) * 64
                                nc.vector.tensor_scalar(
                                    out=hq[:, off:off + fo + 1],
                                    in0=hv_all[hh][jc][:],
                                    scalar1=evqcT[hh][:, 3 * jc:3 * jc + 1],
                                    scalar2=None, op0=OP.mult,
                                )
                            hq3[g][jc] = hq
                # --- phases B1+B2a interleaved: pq sweep for group g, then
                # relu sweeps for its two heads; the next group's pq matmuls
                # overlap the current heads' elementwise masks ---
                # mask-variant schedule per (h, jc): A = all-DVE (ts 4x +
                # tt 2x), B = DVE relu-ts + Pool tt-mult mask, C = DVE
                # tt-mask of w then ACT Relu with bias -rho:
                # relu(w*adj - rho) == relu(w - rho)*adj since rho > 0.
                VAR = "CACBCACBCACBCABC"
                for g in range(4):
                    pqa = acc.tile([P, N], f32, tag="acc")
                    pqp = pqa[0:GROWS, :]
                    for jc in range(NCH):
                        for hf in range(2):
                            nc.tensor.matmul(
                                out=pqp[:, hf * HALF:(hf + 1) * HALF],
                                lhsT=hq3[g][jc][:],
                                rhs=adjT[:, jc * N + hf * HALF:
                                         jc * N + (hf + 1) * HALF],
                                start=(jc == 0), stop=(jc == NCH - 1),
                            )
                    sb = singles.tile([GROWS, N], bf16, tag=f"pqsb_{g}",
                                      name=f"pqsb_{g}")
                    nc.scalar.activation(out=sb[:], in_=pqp[:], func=A.Identity)
                    pq_sb.append(sb)
                    for h in (2 * g, 2 * g + 1):
                        wb = wb_list[h]
                        ev = evqcT[h]
                        evn = evqcTn[h]
                        pata = acc.tile([P, N], f32, tag="acc")
                        pat = pata[0:fo + 1, :]
                        for jc in range(NCH):
                            v = VAR[(h * NCH + jc) % 16]
                            adjc = adjT[:, jc * N:(jc + 1) * N]
                            rho = ev[:, 3 * jc:3 * jc + 1]
                            nrho = evn[:, 3 * jc:3 * jc + 1]
                            mk = big.tile([P, N], bf16, tag="mk", bufs=7)
                            if v == "C":
                                t0 = big.tile([P, N], bf16, tag="rl", bufs=8)
                                nc.vector.tensor_tensor(
                                    out=t0[:], in0=wb[:], in1=adjc, op=OP.mult,
                                )
                                nc.scalar.activation(out=mk[:], in_=t0[:],
                                                     func=A.Relu, bias=nrho)
                            else:
                                rl = big.tile([P, N], bf16, tag="rl", bufs=8)
                                nc.vector.tensor_scalar(
                                    out=rl[:], in0=wb[:],
                                    scalar1=rho, scalar2=0.0,
                                    op0=OP.subtract, op1=OP.max,
                                )
                                if v == "A":
                                    nc.vector.tensor_tensor(
                                        out=mk[:], in0=rl[:], in1=adjc,
                                        op=OP.mult,
                                    )
                                else:
                                    nc.gpsimd.tensor_tensor(
                                        out=mk[:], in0=rl[:], in1=adjc,
                                        op=OP.mult,
                                    )
                            for hf in range(2):
                                fs = slice(hf * HALF, (hf + 1) * HALF)
                                nc.tensor.matmul(
                                    out=pat[:, fs],
                                    lhsT=hv_all[h][jc][:],
                                    rhs=mk[:, fs],
                                    start=(jc == 0), stop=False,
                                )
                        # fold the q-term into pat on the PE via a
                        # shifted-identity lhsT, then evacuate on ACT
                        k = h % 2
                        sel = wpb[0:GROWS, B_SEL2 + k * (fo + 1):
                                  B_SEL2 + (k + 1) * (fo + 1)]
                        for hf in range(2):
                            fs = slice(hf * HALF, (hf + 1) * HALF)
                            nc.tensor.matmul(
                                out=pat[:, fs], lhsT=sel, rhs=pq_sb[g][:, fs],
                                start=False, stop=True,
                            )
                        ou = singles.tile([fo + 1, N], bf16, tag=f"ou_{h}",
                                          name=f"ou_{h}")
                        nc.scalar.activation(out=ou[:], in_=pat[:],
                                             func=A.Identity)
                        ou_list.append(ou)
                        # gather this head's Z row into column form
                        # (zcols col 8h+c <-> (head h, chunk c))
                        pzh3 = pAd.tile([P, 24], f32, tag="pd3")
                        pzh = pzh3[:, 0:H]
                        for c in range(NCH):
                            nc.tensor.matmul(
                                out=pzh[:, c:c + 1],
                                lhsT=ou[fo:fo + 1, c * P:(c + 1) * P],
                                rhs=identb[fo:fo + 1, fo:fo + 1],
                                start=True, stop=True,
                            )
                        nc.vector.tensor_copy(
                            out=zcols[:, H * h:H * h + H], in_=pzh[:]
                        )
            # --- phase B2b/B2c: Z -> 1/Z, normalize, activation epilogue ---
            rall = singles.tile([H, N], f32r, tag="rall", name="rall")
            xr_list = []
            pm = None
            with tc.tile_pool(name=f"pBz{li}", bufs=2, space="PSUM") as pBz, \
                 tc.tile_pool(name=f"pBr{li}", bufs=2, space="PSUM") as pBr, \
                 tc.tile_pool(name=f"pBm{li}", bufs=1, space="PSUM") as pBm:
                rcols = singles.tile([P, H * NCH], f32, tag="rcols",
                                     name="rcols")
                rscr = singles.tile([P, H * NCH], f32, tag="rscr",
                                    name="rscr")
                nc.vector.reciprocal_approx_accurate(
                    out=rcols[:], in_=zcols[:], scratch=rscr[:]
                )
                for c in range(NCH):
                    prr = pBz.tile([H, P], f32, tag="rr")
                    nc.tensor.matmul(
                        out=prr[:],
                        lhsT=rcols[:, c:c + (H - 1) * H + 1:H], rhs=ident,
                        start=True, stop=True,
                    )
                    nc.vector.tensor_copy(
                        out=rall[:, c * P:(c + 1) * P], in_=prr[:]
                    )
                def emit_prb(h):
                    prb = pBr.tile([fo, N], f32, tag="prb")
                    for hf in range(2):
                        fs = slice(hf * HALF, (hf + 1) * HALF)
                        nc.tensor.matmul(
                            out=prb[:, fs],
                            lhsT=wpr[0:H, C_SEL + h * fo: C_SEL + (h + 1) * fo],
                            rhs=rall[:, fs],
                            start=True, stop=True,
                        )
                    return prb

                # lookahead: head h+1's 1/Z broadcast is emitted before head
                # h's dependent ops so the PE queue never stalls on y_h
                prb_next = emit_prb(0)
                for h in range(H):
                    prb = prb_next
                    if h + 1 < H:
                        prb_next = emit_prb(h + 1)
                    y = epi.tile([fo, N], bf16, tag="y", bufs=3)
                    if li == 0:
                        # l0: stage 1/Z through ACT so the DVE multiply
                        # runs in 2x mode (DVE is saturated here)
                        prs = epi.tile([fo, N], bf16, tag="prs", bufs=2)
                        nc.scalar.activation(out=prs[:], in_=prb[:],
                                             func=A.Identity)
                        nc.vector.tensor_tensor(
                            out=y[:], in0=ou_list[h][0:fo, :], in1=prs[:],
                            op=OP.mult
                        )
                    else:
                        # l1 tail: shorter chain, read prb from PSUM
                        nc.vector.tensor_tensor(
                            out=y[:], in0=ou_list[h][0:fo, :], in1=prb[:],
                            op=OP.mult
                        )
                    if li == 0:
                        # x1 rows = elu(y + b0)
                        if not zero_b0:
                            yb = epi.tile([fo, N], bf16, tag="yb")
                            nc.vector.tensor_scalar(
                                out=yb[:], in0=y[:],
                                scalar1=wp[0:fo, C_B0:C_B0 + 1],
                                scalar2=None, op0=OP.add,
                            )
                            y = yb
                        m = epi.tile([fo, N], bf16, tag="m", bufs=2)
                        nc.vector.tensor_scalar(
                            out=m[:], in0=y[:], scalar1=0.0, scalar2=None,
                            op0=OP.min
                        )
                        e = epi.tile([fo, N], bf16, tag="e", bufs=2)
                        nc.scalar.activation(out=e[:], in_=m[:], func=A.Exp)
                        xr = epi.tile([fo, N], bf16, tag="xr", bufs=8,
                                      name=f"xr{h}")
                        nc.vector.scalar_tensor_tensor(
                            out=xr[:], in0=e[:], scalar=-1.0, in1=y[:],
                            op0=OP.add, op1=OP.max,
                        )
                        xr_list.append(xr)
                        # restack half as soon as its 4 heads are done
                        if h % 4 == 3:
                            k = h // 4
                            px1 = pBm.tile([P, N], f32, tag="mx",
                                           name=f"px1_{k}")
                            for hf in range(2):
                                fs = slice(hf * HALF, (hf + 1) * HALF)
                                for j in range(4):
                                    nc.tensor.matmul(
                                        out=px1[:, fs],
                                        lhsT=wpb[0:fo, B_SBLK + j * P:
                                                 B_SBLK + (j + 1) * P],
                                        rhs=xr_list[k * 4 + j][:, fs],
                                        start=(j == 0), stop=(j == 3),
                                    )
                            nc.scalar.activation(out=x1T[k][:], in_=px1[:],
                                                 func=A.Identity)
                    else:
                        # head-mean accumulation: pm += mw^T @ y
                        if pm is None:
                            pm = pBm.tile([16, N], f32, tag="mx")
                        for hf in range(2):
                            fs = slice(hf * HALF, (hf + 1) * HALF)
                            nc.tensor.matmul(
                                out=pm[:, fs],
                                lhsT=wpb[0:fo, B_MW:B_MW + 16],
                                rhs=y[:, fs],
                                start=(h == 0), stop=(h == H - 1),
                            )
                if li == 0:
                    xT_in = x1T
                else:
                    # rows 0:16 = pm + b1; row 32 later holds logZ
                    msbt = singles.tile([33, N], f32, tag="msb")
                    msb = msbt[0:16, :]
                    nc.vector.tensor_scalar(
                        out=msb[:], in0=pm[:], scalar1=wp[0:16, C_B1:C_B1 + 1],
                        scalar2=None, op0=OP.add,
                    )

        # ---- log_softmax over the 16 features ----
        # feature-major: one exp + ones-column colsum (PE) + one ln, then
        # per-chunk transposes carry both msb^T and logZ^T; final subtract
        # is a per-partition-scalar tensor_scalar. Scores are bounded, so
        # no max-subtraction is needed.
        with tc.tile_pool(name="pfin", bufs=2, space="PSUM") as pfin, \
             tc.tile_pool(name="pfz", bufs=1, space="PSUM") as pfz, \
             tc.tile_pool(name="finsb", bufs=1) as finsb:
            pexp = finsb.tile([16, N], bf16, tag="pexp")
            nc.scalar.activation(out=pexp[:], in_=msb[:], func=A.Exp)
            pzs = pfz.tile([1, N], f32, tag="zs")
            for hf in range(2):
                fs = slice(hf * HALF, (hf + 1) * HALF)
                nc.tensor.matmul(
                    out=pzs[:, fs], lhsT=wpb[0:16, B_ONE:B_ONE + 1],
                    rhs=pexp[:, fs], start=True, stop=True,
                )
            lgz = msbt[32:33, :]
            nc.scalar.activation(out=lgz[:], in_=pzs[:], func=A.Ln)
            outall = finsb.tile([P, NCH * 16], f32, tag="outall")
            for ic in range(NCH):
                pf = pfin.tile([P, 17], f32, tag="sm")
                nc.tensor.matmul(
                    out=pf[:, 0:16], lhsT=msb[:, ic * P:(ic + 1) * P],
                    rhs=wp[0:16, 0:16],
                    start=True, stop=True,
                )
                nc.tensor.matmul(
                    out=pf[:, 16:17], lhsT=lgz[:, ic * P:(ic + 1) * P],
                    rhs=wp[32:33, 32:33],
                    start=True, stop=True,
                )
                res = respool.tile([P, 17], f32, tag="res")
                nc.vector.tensor_copy(out=res[:], in_=pf[:])
                nc.vector.tensor_scalar(
                    out=outall[:, 16 * ic:16 * (ic + 1)], in0=res[:, 0:16],
                    scalar1=res[:, 16:17],
                    scalar2=None, op0=OP.subtract,
                )
            # single contiguous output DMA; host undoes the layout
            nc.sync.dma_start(out=out_d[:, :], in_=outall[:])

    nc.compile()
    return nc


def _make_wpack(inputs):
    f32 = np.float32
    import ml_dtypes
    wpack = np.zeros((P, WCOLS), f32)
    wpack[:, C_IDENT:C_IDENT + P] = np.eye(P, dtype=f32)
    wpack[0:FO, C_B0] = np.asarray(inputs["b0"], f32).reshape(FO)
    wpack[0:16, C_B1] = np.asarray(inputs["b1"], f32).reshape(16)

    wpr = np.zeros((P, RCOLS), f32)
    w0 = np.asarray(inputs["w0"], f32)
    for h in range(H):
        wpr[:, C_W0 + h * FO: C_W0 + (h + 1) * FO] = w0[h]
    a_src0 = np.asarray(inputs["a_src0"], f32)[..., 0]  # [8, 32]
    a_dst0 = np.asarray(inputs["a_dst0"], f32)[..., 0]
    a_src1 = np.asarray(inputs["a_src1"], f32)[..., 0]  # [8, 16]
    a_dst1 = np.asarray(inputs["a_dst1"], f32)[..., 0]
    for h in range(H):
        wpr[0:FO, C_AS + h] = a_src0[h]
        wpr[0:16, C_AS + H + h] = a_src1[h]
        for k, c in enumerate((-0.8, 1.0, 0.2)):
            wpr[0:FO, C_AD3 + 3 * h + k] = c * a_dst0[h]
            wpr[0:16, C_AD3 + 3 * (H + h) + k] = c * a_dst1[h]
    wpr[0:H, C_SEL:C_SEL + H * FO] = np.kron(
        np.eye(H, dtype=f32), np.ones((1, FO), f32)
    )

    wpb = np.zeros((P, BCOLS), f32)
    wpb[:, B_IDENT:B_IDENT + P] = np.eye(P, dtype=f32)
    w1 = np.asarray(inputs["w1"], f32)  # [8, 256, 16]
    for h in range(H):
        for k in range(2):
            blk = np.zeros((P, FO), f32)
            blk[:, :16] = w1[h, k * P:(k + 1) * P, :]
            wpb[:, B_W1 + (h * 2 + k) * FO: B_W1 + (h * 2 + k + 1) * FO] = blk
    wpb[0:16, B_MW:B_MW + 16] = np.eye(16, dtype=f32) / 8.0
    wpb[0:16, B_ONE] = 1.0
    for k in range(2):
        blk = np.zeros((P, FO + 1), f32)
        for p in range(FO + 1):
            blk[64 * k + p, p] = 1.0
        wpb[:, B_SEL2 + k * (FO + 1): B_SEL2 + (k + 1) * (FO + 1)] = blk
    for j in range(4):
        wpb[0:FO, B_SBLK + j * P: B_SBLK + (j + 1) * P] = np.eye(
            FO, P, k=j * FO, dtype=f32
        )
    wpb = wpb.astype(ml_dtypes.bfloat16)
    return wpack, wpr, wpb


def _prep_inputs(inputs):
    x = np.asarray(inputs["x"], np.float32)
    verts = np.asarray(inputs["vertices"]).astype(np.int32)
    adj = np.asarray(inputs["adj"]).astype(np.uint8)
    emb_w = np.ascontiguousarray(np.asarray(inputs["emb_w"], np.float32))
    wpack, wpr, wpb = _make_wpack(inputs)
    wpack = np.ascontiguousarray(wpack)
    wpr = np.ascontiguousarray(wpr)
    wpb = np.ascontiguousarray(wpb)
    import ml_dtypes
    onesrow = np.ascontiguousarray(np.ones((1, N), dtype=ml_dtypes.bfloat16))
    in_maps = []
    for c in range(BS):
        in_maps.append({
            "x": np.ascontiguousarray(
                x[c].reshape(NCH, P, FEAT).transpose(1, 0, 2).reshape(P, NCH * FEAT)
            ),
            "verts": np.ascontiguousarray(
                verts[c].reshape(NCH, P).T
            ),
            "adj": np.ascontiguousarray(adj[c]),
            "emb_w": emb_w,
            "wpack": wpack,
            "wpackb": wpb,
            "wpackr": wpr,
            "onesrow": onesrow,
        })
    zero_b0 = bool(np.all(np.asarray(inputs["b0"]) == 0))
    return in_maps, zero_b0


def _run(inputs, trace=False):
    from concourse.bass_utils import run_bass_kernel_spmd

    in_maps, zero_b0 = _prep_inputs(inputs)
    key = ("prog", zero_b0)
    if key not in _CACHE:
        _CACHE[key] = _build(zero_b0)
    nc = _CACHE[key]
    res = run_bass_kernel_spmd(
        nc, in_maps, list(range(BS)), trace=trace
    )
    outs = []
    for c in range(BS):
        o = np.asarray(res.results[c]["out"]).reshape(P, NCH, 16)
        outs.append(o.transpose(1, 0, 2).reshape(N, 16))
    out = np.stack(outs, axis=0)
    return out.astype(np.float32), res


def kernel(**inputs):
    out, _ = _run(inputs, trace=False)
    return out
